# revision 32
# baseline (speedup 1.0000x reference)
"""Trainium2 Bass kernel for nn_ContrastiveLoss (N=4096, D=128, NT=512, Q=8).

Strategy (8 NeuronCores, data parallel over N, no cross-core collective):
  - Each core owns R = N/8 = 512 rows of x (4 chunks of 128 partitions).
  - Host pre-transposes x and yf (=y.reshape(N,D)) to bf16 so both matmul
    operands load contiguously as [D=128 partitions, N] tiles.
  - Per row-chunk the core computes S_xy = x_chunk @ yf.T and S_xx =
    x_chunk @ x.T on the PE in bf16 (K=D=128), exp(S/T) on the ACT engine
    with the fused per-partition accumulator giving the row sums.
  - den_x (same-track-excluded sum) comes from one fused DVE
    scalar_tensor_tensor per exp tile: (tcol != trow) * E, accumulated.
  - Positive-pair path: host-gathered y[track[i]] views, fused multiply +
    reduce dots on DVE; sim_p = min of the 8 dots, num = exp(sim_p/T).
  - The pair term SUM_ij log(den_j + num_i) is factored through the log1p
    series (num_i/den_j << 1 for normalized embeddings):
      N*SUM_j log den_j + SUM_k (-1)^(k+1)/k (SUM_i num_i^k)(SUM_j den_j^-k)
    so each core only emits 40 partial moments — no AllGather (which costs
    ~65us of cross-core sync on this runtime). The host checks the series
    tail and falls back to an exact numpy evaluation if it ever fails to
    converge (u >= ~1), which cannot happen for unit-norm inputs
    (num <= e^{1/T} always, den > 0 grows with N).
"""

import numpy as np
import ml_dtypes

import concourse.bass as bass
import concourse.bacc as bacc
import concourse.tile as tile
import concourse.mybir as mybir
from concourse import bass_utils

P = 128           # partitions / rows per chunk
N = 4096          # total rows of x
D = 128           # feature dim
NT = 512          # number of tracks
Q = 8             # views per track
CORES = 8
R = N // CORES    # rows per core = 512
NCH = R // P      # chunks per core = 4
TEMP = 0.05
INV_T = 1.0 / TEMP
HALF = 2048       # exp tile width (4 PSUM banks)
NH = N // HALF    # 2 halves
K_SER = 4         # log1p series order emitted by the device
NGRP = 2 * K_SER + 2          # logden, num^1..4, r^1..4, simp
RED_C = NGRP * NCH            # 40 output columns
F32 = mybir.dt.float32
BF16 = mybir.dt.bfloat16
AX = mybir.AxisListType
ALU = mybir.AluOpType
ACTF = mybir.ActivationFunctionType

_CACHE = {}

# Force the activation-table chooser onto the combined exp+ln set: blank the
# contents of every other set (indices must stay aligned with act_info.json)
# so only one ACT_TABLE_LOAD is ever emitted.
import concourse.bacc as _bacc_mod
import concourse.hw_specs as _hw_specs

_orig_get_tables = _hw_specs.get_activation_tables


def _combined_only_tables(module_arch):
    full = _orig_get_tables(module_arch)
    return {name: (funcs if name == "natural_log_exp_and_others" else set())
            for name, funcs in full.items()}


_bacc_mod.get_activation_tables = _combined_only_tables


def _build():
    nc = bacc.Bacc("TRN2", target_bir_lowering=False, debug=False,
                   num_devices=CORES)

    xT_d = nc.dram_tensor("xT", [D, N], BF16, kind="ExternalInput")
    yT_d = nc.dram_tensor("yT", [D, N], BF16, kind="ExternalInput")
    xsh_d = nc.dram_tensor("xsh", [D, R], BF16, kind="ExternalInput")
    # xrow: per-core natural-layout rows, [:, 128*cc + d] = x[...]
    xrow_d = nc.dram_tensor("xrow", [P, R], BF16, kind="ExternalInput")
    # trow: per-chunk track ids, [p, cc] = track[512*c + 128*cc + p]
    trow_d = nc.dram_tensor("trow", [P, NCH], F32, kind="ExternalInput")
    # yown: host-gathered positive views, [p, 1024*cc + 128*q + d]
    #   = y[track[512*c + 128*cc + p], q, d]
    yown_d = nc.dram_tensor("yown", [P, NCH * Q * D], BF16,
                            kind="ExternalInput")
    # track id of every x column (broadcast across partitions on device)
    tcol_d = nc.dram_tensor("tcol", [1, N], BF16, kind="ExternalInput")
    out_d = nc.dram_tensor("out", [1, RED_C], F32, kind="ExternalOutput")

    with tile.TileContext(nc) as tc:
        with (
            tc.tile_pool(name="persist", bufs=1) as pp,
            tc.tile_pool(name="escr", bufs=3) as ep,
            tc.tile_pool(name="sttjunk", bufs=2) as sjp,
            tc.tile_pool(name="ttrjunk", bufs=2) as tjp,
            tc.tile_pool(name="eown", bufs=2) as eop,
            tc.tile_pool(name="psum", bufs=2, space="PSUM") as psp,
        ):
            # ---- persistent tiles ----
            xT_s = pp.tile([D, N], BF16, tag="xT_s")
            yT_s = pp.tile([D, N], BF16, tag="yT_s")
            xsh_s = pp.tile([D, R], BF16, tag="xsh_s")
            xrow_s = pp.tile([P, R], BF16, tag="xrow_s")
            trow_s = pp.tile([P, NCH], F32, tag="trow_s")
            yown_s = pp.tile([P, NCH * Q * D], BF16, tag="yown_s")
            tcol_s = pp.tile([P, N], BF16, tag="tcol_s")
            dots_s = pp.tile([P, NCH * Q], F32, tag="dots_s")
            simp_s = pp.tile([P, NCH], F32, tag="simp_s")
            own_s = pp.tile([P, NCH], F32, tag="own_s")
            ones_s = pp.tile([P, 1], F32, tag="ones_s")
            toty_s = pp.tile([P, 2 * NCH], F32, tag="toty_s")
            denx_s = pp.tile([P, 2 * NCH], F32, tag="denx_s")
            toty4_s = pp.tile([P, NCH], F32, tag="toty4_s")
            denx4_s = pp.tile([P, NCH], F32, tag="denx4_s")
            den_s = pp.tile([P, NCH], F32, tag="den_s")
            redv_s = pp.tile([P, RED_C], F32, tag="redv_s")
            outr_s = pp.tile([P, RED_C], F32, tag="outr_s")

            # redv column groups
            LD, NU, RC, SI = 0, NCH, NCH * (1 + K_SER), NCH * (1 + 2 * K_SER)

            # ---- input loads (order matters for queue priority) ----
            nc.sync.dma_start(out=xsh_s[:], in_=xsh_d.ap())
            # xx tiles run first in each chunk: land xT[0:2048] earliest
            for k in range(2):
                sl = slice(k * (N // 4), (k + 1) * (N // 4))
                nc.sync.dma_start(out=xT_s[:, sl], in_=xT_d.ap()[:, sl])
            for k in range(2):
                sl = slice(k * (N // 4), (k + 1) * (N // 4))
                nc.sync.dma_start(out=yT_s[:, sl], in_=yT_d.ap()[:, sl])
            for k in range(2, 4):
                sl = slice(k * (N // 4), (k + 1) * (N // 4))
                nc.sync.dma_start(out=xT_s[:, sl], in_=xT_d.ap()[:, sl])
                nc.sync.dma_start(out=yT_s[:, sl], in_=yT_d.ap()[:, sl])
            # off-critical loads on the idle SWDGE queues
            nc.gpsimd.dma_start(out=xrow_s[:], in_=xrow_d.ap())
            nc.gpsimd.dma_start(out=trow_s[:], in_=trow_d.ap())
            nc.gpsimd.dma_start(out=yown_s[:], in_=yown_d.ap())
            # broadcast the column track-ids across all 128 partitions
            for k in range(2):
                sl = slice(k * (N // 2), (k + 1) * (N // 2))
                nc.gpsimd.dma_start(
                    out=tcol_s[:, sl],
                    in_=tcol_d.ap()[0:1, sl].to_broadcast([P, N // 2]))

            # ---- positive-pair path: dots -> sim_p -> num moments ----
            for cc in range(NCH):
                xrep = (xrow_s[:, cc * D:(cc + 1) * D]
                        .rearrange("p (o d) -> p o d", o=1)
                        .to_broadcast([P, Q, D]))
                yo = yown_s[:, cc * Q * D:(cc + 1) * Q * D]
                tj = tjp.tile([P, Q * D], BF16, tag="ttrjunk")
                nc.vector.tensor_tensor(
                    out=tj[:].rearrange("p (q d) -> p q d", d=D),
                    in0=yo.rearrange("p (q d) -> p q d", d=D),
                    in1=xrep,
                    op=ALU.mult,
                )
                nc.vector.tensor_reduce(
                    out=dots_s[:, Q * cc:Q * (cc + 1)],
                    in_=tj[:].rearrange("p (q d) -> p q d", d=D),
                    axis=AX.X, op=ALU.add,
                )
                nc.vector.tensor_reduce(
                    out=simp_s[:, cc:cc + 1],
                    in_=dots_s[:, Q * cc:Q * (cc + 1)],
                    axis=AX.X, op=ALU.min,
                )
                eo = eop.tile([P, Q], F32, tag="eown")
                nc.scalar.activation(
                    out=eo[:],
                    in_=dots_s[:, Q * cc:Q * (cc + 1)],
                    func=ACTF.Exp,
                    scale=INV_T,
                    accum_out=own_s[:, cc:cc + 1],
                )
            # num^k = exp(k * sim_p / T) straight from the ACT affine stage
            for k in range(1, K_SER + 1):
                nc.scalar.activation(
                    out=redv_s[:, NU + (k - 1) * NCH:NU + k * NCH],
                    in_=simp_s[:], func=ACTF.Exp, scale=k * INV_T)

            nc.vector.memset(ones_s[:], 1.0)
            # dummy activation: pulls the exp/ln table load into the DMA head
            warm = eop.tile([P, Q], F32, tag="eown")
            nc.scalar.activation(out=warm[:], in_=ones_s[:].to_broadcast([P, Q]),
                                 func=ACTF.Exp, scale=1.0)

            # ---- big matmuls + exp + fused reductions ----
            for cc in range(NCH):
                lhsT = xsh_s[:, cc * P:(cc + 1) * P]
                for h in range(NH):
                    # --- xx ---
                    ps2 = psp.tile([P, HALF], F32, tag="ps")
                    for k in range(HALF // 512):
                        nc.tensor.matmul(
                            out=ps2[:, 512 * k:512 * (k + 1)],
                            lhsT=lhsT,
                            rhs=xT_s[:, HALF * h + 512 * k:HALF * h + 512 * (k + 1)],
                            start=True, stop=True,
                        )
                    e2 = ep.tile([P, HALF], BF16, tag="escr2")
                    nc.scalar.activation(
                        out=e2[:], in_=ps2[:], func=ACTF.Exp, scale=INV_T,
                    )
                    sj = sjp.tile([P, HALF], BF16, tag="sttjunk")
                    nc.vector.scalar_tensor_tensor(
                        out=sj[:],
                        in0=tcol_s[:, HALF * h:HALF * (h + 1)],
                        scalar=trow_s[:, cc:cc + 1],
                        in1=e2[:],
                        op0=ALU.not_equal,
                        op1=ALU.mult,
                        accum_out=denx_s[:, 2 * cc + h:2 * cc + h + 1],
                    )
                    # --- xy ---
                    ps = psp.tile([P, HALF], F32, tag="ps")
                    for k in range(HALF // 512):
                        nc.tensor.matmul(
                            out=ps[:, 512 * k:512 * (k + 1)],
                            lhsT=lhsT,
                            rhs=yT_s[:, HALF * h + 512 * k:HALF * h + 512 * (k + 1)],
                            start=True, stop=True,
                        )
                    nc.scalar.activation(
                        out=ps[:], in_=ps[:], func=ACTF.Exp, scale=INV_T,
                        accum_out=toty_s[:, 2 * cc + h:2 * cc + h + 1],
                    )
                # den_cc = (toty_cc - own_cc) + denx_cc, as soon as ready
                nc.vector.tensor_reduce(
                    out=toty4_s[:, cc:cc + 1],
                    in_=toty_s[:, 2 * cc:2 * cc + 2],
                    axis=AX.X, op=ALU.add,
                )
                nc.vector.tensor_reduce(
                    out=denx4_s[:, cc:cc + 1],
                    in_=denx_s[:, 2 * cc:2 * cc + 2],
                    axis=AX.X, op=ALU.add,
                )
                nc.vector.tensor_tensor(
                    out=den_s[:, cc:cc + 1], in0=toty4_s[:, cc:cc + 1],
                    in1=own_s[:, cc:cc + 1], op=ALU.subtract)
                nc.vector.tensor_tensor(
                    out=den_s[:, cc:cc + 1], in0=den_s[:, cc:cc + 1],
                    in1=denx4_s[:, cc:cc + 1], op=ALU.add)

            # ---- per-core moments + output ----
            nc.scalar.activation(out=redv_s[:, LD:LD + NCH], in_=den_s[:],
                                 func=ACTF.Ln)
            r1 = redv_s[:, RC:RC + NCH]
            r2 = redv_s[:, RC + NCH:RC + 2 * NCH]
            r3 = redv_s[:, RC + 2 * NCH:RC + 3 * NCH]
            r4 = redv_s[:, RC + 3 * NCH:RC + 4 * NCH]
            nc.vector.reciprocal(r1, den_s[:])
            nc.vector.tensor_tensor(out=r2, in0=r1, in1=r1, op=ALU.mult)
            nc.vector.tensor_tensor(out=r3, in0=r2, in1=r1, op=ALU.mult)
            nc.vector.tensor_tensor(out=r4, in0=r2, in1=r2, op=ALU.mult)
            nc.vector.tensor_copy(redv_s[:, SI:SI + NCH], simp_s[:])

            # partition-axis reduction: ones.T @ redv on the PE
            psr = psp.tile([P, HALF], F32, tag="ps")
            nc.tensor.matmul(out=psr[0:1, 0:RED_C], lhsT=ones_s[:],
                             rhs=redv_s[:], start=True, stop=True)
            nc.vector.tensor_copy(outr_s[0:1, :], psr[0:1, 0:RED_C])
            nc.sync.dma_start(out=out_d.ap(), in_=outr_s[0:1, :])

    nc.compile()
    return nc


def get_nc():
    if "nc" not in _CACHE:
        _CACHE["nc"] = _build()
    return _CACHE["nc"]


def prepare_in_maps(x, track_idxs, y):
    x = np.ascontiguousarray(np.asarray(x), dtype=np.float32)
    y = np.ascontiguousarray(np.asarray(y), dtype=np.float32)
    t = np.asarray(track_idxs).astype(np.int64)
    xT = np.ascontiguousarray(x.T.astype(ml_dtypes.bfloat16))
    yT = np.ascontiguousarray(y.reshape(N, D).T.astype(ml_dtypes.bfloat16))
    tf = t.astype(np.float32)
    tcol = np.ascontiguousarray((tf - 256.0).reshape(1, N).astype(ml_dtypes.bfloat16))
    in_maps = []
    for c in range(CORES):
        rows = slice(c * R, (c + 1) * R)
        xsh = np.ascontiguousarray(xT[:, rows])
        # natural-layout rows, chunk-major: [p, 128*cc + d]
        xrow = (x[rows].reshape(NCH, P, D).transpose(1, 0, 2)
                .reshape(P, R).astype(ml_dtypes.bfloat16))
        trow = np.ascontiguousarray(
            (tf[rows] - 256.0).reshape(NCH, P).T)
        # y views of each row's own track: [512, 8, 128] -> [128, 4*1024]
        yo = (y[t[rows]].reshape(NCH, P, Q * D)
              .transpose(1, 0, 2).reshape(P, NCH * Q * D)
              .astype(ml_dtypes.bfloat16))
        in_maps.append({
            "xT": xT, "yT": yT, "xsh": xsh,
            "xrow": np.ascontiguousarray(xrow),
            "trow": trow,
            "yown": np.ascontiguousarray(yo),
            "tcol": tcol,
        })
    return in_maps


def _exact_fallback(x, track_idxs, y):
    x = np.asarray(x, dtype=np.float64)
    y = np.asarray(y, dtype=np.float64)
    t = np.asarray(track_idxs)
    yf = y.reshape(NT * Q, D)
    ct = np.repeat(np.arange(NT), Q)
    own = t[:, None] == ct[None, :]
    S_xy = x @ yf.T
    sim_p = np.where(own, S_xy, np.inf).min(1)
    num = np.exp(sim_p / TEMP)
    den_y = np.where(own, 0.0, np.exp(S_xy / TEMP)).sum(1)
    same = t[:, None] == t[None, :]
    S_xx = x @ x.T
    den_x = np.where(same, 0.0, np.exp(S_xx / TEMP)).sum(1)
    den = den_y + den_x
    loss = np.log(den[None, :] + num[:, None]).mean() - (sim_p / TEMP).mean()
    return np.float32(loss)


def combine_outputs(outs, inputs=None):
    """outs: per-core [1, 40] moment arrays -> scalar loss via the log1p
    series. Falls back to exact evaluation if the series is not safely
    convergent (requires `inputs`)."""
    g = np.zeros(NGRP, dtype=np.float64)
    ok = True
    for o in outs:
        o = np.asarray(o, dtype=np.float64).reshape(NGRP, NCH)
        if not np.all(np.isfinite(o)):
            ok = False
            break
        g += o.sum(axis=1)
    if ok:
        logden, simp = g[0], g[NGRP - 1]
        nums = g[1:1 + K_SER]
        rinv = g[1 + K_SER:1 + 2 * K_SER]
        terms = [(-1.0) ** (k + 1) / k * nums[k - 1] * rinv[k - 1]
                 for k in range(1, K_SER + 1)]
        pair = N * logden + sum(terms)
        # series tail must be vanishing for the truncation to be valid
        if not (abs(terms[-1]) <= 1e-8 * abs(pair) + 1e-12
                and abs(terms[-1]) <= abs(terms[-2]) + 1e-30):
            ok = False
        else:
            return np.float32(pair / (N * N) - simp * INV_T / N)
    if inputs is None:
        raise RuntimeError("series check failed and no inputs for fallback")
    return _exact_fallback(**inputs)


def kernel(x, track_idxs, y):
    nc = get_nc()
    in_maps = prepare_in_maps(x, track_idxs, y)
    res = bass_utils.run_bass_kernel_spmd(nc, in_maps,
                                          core_ids=list(range(CORES)))
    return combine_outputs([r["out"] for r in res.results],
                           inputs={"x": x, "track_idxs": track_idxs, "y": y})


if __name__ == "__main__":
    nc = get_nc()
    print("build + compile OK")
